# revision 27
# baseline (speedup 1.0000x reference)
"""Trainium2 Bass kernel for AdaptiveDiffAttention.

Pure data parallel across 8 NeuronCores: each core processes B/8 = 2048
samples with a replicated copy of the weights. No collectives.

Math per sample b (seq len 2, heads 4, head dim 256):
  tokens  = x.reshape(2, 1024)
  lam     = sigmoid(relu(x @ L1) @ L2)
  Q,K,V   = tokens @ W*  (per token)
  softmax over 2 keys => a_q = sigmoid(SCALE * <Q_q, K_0 - K_1>)
  w0_q = relu(a1_q - lam*a2_q); w1_q = relu((1-a1_q) - lam*(1-a2_q))
  A_q  = w0_q * V_0 + w1_q * V_1   (per head)
  out_q = A_q @ WO + tokens_q

All big GEMMs run as fp8e4 DoubleRow matmuls. Scale management (TRN
e4m3 normal range [2^-6, 240]):
  x8 = 16*x, W' = 64*W  =>  Q'/Kd' psum = 1024*(Q/Kd)
  score sigmoid folds SCALE/2^20; V evicted with 1/1024 to natural bf16;
  A evicted with x8 to fp8 (stationary for the fp8 WO matmul);
  WO' = 64*WO => psum = 512*out_attn; o = psum/512 + x (bf16 residual).

Engine balance (v2): the v1 kernel was vector-engine-bound (~21us/tile
DVE busy vs ~23us PE). v2 rebalances so the PE is the only bottleneck:
  - x^T and the token difference (T0-T1)^T are precomputed on host and
    shipped fp8 (no on-chip transposes or subtracts).
  - Q projections are never evicted: the per-head score dot products run
    as fused DVE scalar_tensor_tensor (mult+mult+accum) reading the Q
    psum directly (GPSIMD/Pool cannot access PSUM, and its per-op
    dispatch overhead measured ~17us/tile — everything elementwise stays
    on DVE/ACT).
  - Kd/V/A^T psum evictions on ACT; the WO eviction + residual add are a
    single DVE scalar_tensor_tensor; lam's relu+dot folds into one DVE
    stt with accumulate ((psum max 0) * L2, accum).
  - x ships bf16 (residual precision is ample; halves that DMA).
  - The attention combine runs as fp8 DoubleRow diag-matmuls: V evicted
    as 16*V fp8 with V_0/V_1 kv-interleaved so one stationary covers
    both keys (16 MMs + 8 LDWEIGHTS instead of 32 + 32; costs ~+0.2e-2
    rel err, total ~1.25e-2 vs the 2e-2 gate).
  - LDWEIGHTS is expensive when exposed (~213ns per DoubleRow load, and
    walrus's ldw-opt is disabled and hard-fails when enabled), so every
    GEMM loop keeps the stationary operand outermost: kd1+kd2 share the
    xtd chunk stationaries, q1+q2+lam share the x^T chunk stationaries
    (5 MMs per load), and WO shares each A^T chunk across both N halves.
Software pipelining: each tile is split into phase A (projection GEMMs,
scores, lam, dmats) and phase B (combine + WO + output).  Emission order
A(0) A(1) B(0) A(2) B(1) ... so the PE stream of tile t+1's projections
covers the DVE score->dmat chain of tile t; the PE never waits on the
post-GEMM pipeline (measured +10.6us/tile without it).  PSUM granularity
is [128,512] (one bank): 4 proj bufs + 2 lam + 2 combine/WO = 8 banks.

Startup: weight DMAs are emitted in first-use order (K1/K2 first, WO
last) and tile-0's x^T DMA is hoisted ahead of them, so the first
matmul waits on ~1.4 MB of transfers rather than all ~6.9 MB of weights.
"""

import contextlib
import sys

for _p in ("/opt/trn_rl_repo", "/root/.axon_site/_ro/trn_rl_repo"):
    if _p not in sys.path:
        sys.path.append(_p)

import numpy as np
import ml_dtypes

import concourse.bass as bass
import concourse.mybir as mybir
import bass_rust
from concourse.tile import TileContext
from concourse.masks import make_identity

F32 = mybir.dt.float32
BF16 = mybir.dt.bfloat16
FP8 = mybir.dt.float8e4

DIM = 2048
SD = 1024
H = 4
HD = 256
LH = 256
SCALE = HD ** -0.5
N_CORES = 8
B_FULL = 16384
B_CORE = B_FULL // N_CORES  # 2048

X_SCALE = 16.0
W_SCALE = 64.0
QK_SCALE = X_SCALE * W_SCALE          # 1024: Q'/Kd'/V' = 1024 * natural
A_SCALE = 8.0                          # at' = 8*A in fp8
O_SCALE = A_SCALE * W_SCALE            # 512: WO psum = 512 * out_attn

AluOp = mybir.AluOpType
ActFn = mybir.ActivationFunctionType
DR = mybir.MatmulPerfMode.DoubleRow


def split_excess_waits(nc, max_waits=1):
    """Walrus codegen in this container rejects >1 sync wait on CTRL-class
    instructions. Move excess waits onto chained nops before the offender."""
    for f in nc.m.functions:
        for bb in f.blocks:
            new_insts = []
            for inst in bb.instructions:
                si = inst.sync_info
                if si is not None and si.on_wait and len(si.on_wait) > max_waits:
                    waits = list(si.on_wait)
                    extra, keep = waits[:-max_waits], waits[-max_waits:]
                    for ci in range(0, len(extra), max_waits):
                        chunk = extra[ci:ci + max_waits]
                        nop = mybir.InstNoOp(name=f"{inst.name}-wsplit{ci}")
                        nop.engine = inst.engine
                        nop.sync_info = bass_rust.SyncInfo(
                            on_wait=chunk, on_update=[])
                        nc.register_instruction(nop, overwrite=True)
                        new_insts.append(nop)
                    inst.sync_info = bass_rust.SyncInfo(
                        on_wait=keep, on_update=list(si.on_update or []))
                new_insts.append(inst)
            bb.instructions = new_insts


DEFAULT_VARIANT = dict(
    pool_ops=False,     # dmats + small weight math on Pool (else DVE).
                        # False: GPSIMD per-op dispatch measured ~17us/tile.
    score_mode="stt",   # "stt": fused DVE stt from psum; "evict": v1-style
    pipeline=True,      # A/B software pipelining
    ldw_share=True,     # stationary-outer loop order in GEMMs
    fuse_proj=True,     # kd1+kd2 / q1+q2+lam share stationary x^T chunks
    nob=False,          # timing bisect: skip phase B entirely
    nodmat=False,       # timing bisect: combine reads identity, no dmat ops
    psl2=True,          # psA=4 + psL=2 (lam psum double buffered)
    vride=False,        # V MMs ride the q-pass stationaries (7 MM / LDW);
                        # single shared 7-buf psum ring for A and B
    c8=True,            # combine via fp8 DoubleRow: V0/V1 stacked stationary
                        # (16*V fp8), stacked diag weights; 16 MMs / 8 LDW
    bmix=False,         # interleave combine(q1) MMs into wo(q0) stream so
                        # combine LDWEIGHTS hide under 512-col wo streams
)


def build_kernel(n_samples=B_CORE, repeats=1, hw_repeats=1, variant=None):
    """Build the single-core Bass graph. n_samples must be a multiple of 128.

    hw_repeats: hardware For_i loop around the whole tile loop (graph does
    not grow) — used for timing with large in-NEFF repeat factors."""
    v = dict(DEFAULT_VARIANT)
    if variant:
        v.update(variant)
    assert n_samples % 128 == 0
    n_mtiles = n_samples // 128

    nc = bass.Bass()

    # x in bf16 (residual only); x^T per m-tile in fp8 (16*x):
    # [mt, feat_in_tile(p), ftile, b] with ftile 0..15 = x^T, 16..23 = xtd^T
    x_d = nc.declare_dram_parameter("xb", [n_samples, DIM], BF16,
                                    isOutput=False)
    xtp_d = nc.declare_dram_parameter(
        "xtp", [n_mtiles, 128, 24, 128], FP8, isOutput=False)
    w_d = {}
    for name, pname in (("q1", "WQ1_w"), ("k1", "WK1_w"), ("q2", "WQ2_w"),
                        ("k2", "WK2_w"), ("v", "WV_w"), ("o", "WO_w")):
        w_d[name] = nc.declare_dram_parameter(pname, [SD, SD], FP8,
                                              isOutput=False)
    l1_d = nc.declare_dram_parameter("L1_w", [DIM, LH], FP8, isOutput=False)
    l2r_d = nc.declare_dram_parameter("L2r", [128, LH], F32, isOutput=False)
    out_d = nc.declare_dram_parameter("out", [n_samples, DIM], F32,
                                      isOutput=True)

    with TileContext(nc) as tc:
        with (
            tc.tile_pool(name="const", bufs=1) as const,
            tc.tile_pool(name="xnat", bufs=2) as xnat_p,
            tc.tile_pool(name="xt", bufs=2) as xt_p,
            tc.tile_pool(name="kdp", bufs=2) as kd_p,
            tc.tile_pool(name="vbuf", bufs=2) as v_p,
            tc.tile_pool(name="scr", bufs=2) as scr_p,
            tc.tile_pool(name="small", bufs=2) as small_p,
            tc.tile_pool(name="hbuf", bufs=2) as h_p,
            tc.tile_pool(name="dpool", bufs=32) as d_p,
            tc.tile_pool(name="at", bufs=2) as at_p,
            tc.tile_pool(name="obuf", bufs=8) as o_p,
            tc.tile_pool(name="psA", bufs=(6 if v["psl2"] else 7)
                         if v["vride"] else (4 if v["psl2"] else 5),
                         space="PSUM") as psA,
            tc.tile_pool(name="psL", bufs=2 if v["psl2"] else 1,
                         space="PSUM") as psL,
            tc.tile_pool(name="psB", bufs=2, space="PSUM") as psBpool,
        ):
            # ---------------- resident weights (already fp8, x64) -----------
            w_sb = {}
            for name in ("k1", "k2", "q1", "q2", "v", "o"):
                wt = const.tile([128, 8, SD], FP8, name=f"w_{name}")
                w_sb[name] = wt
            l1_sb = const.tile([128, 16, LH], FP8, name="l1")
            l2_rep = const.tile([128, LH], F32, name="l2rep")
            id_bf16 = const.tile([128, 128], BF16, name="id16")

            def wdma(name):
                wr = w_d[name].rearrange("(ko p) n -> p ko n", p=128)
                nc.sync.dma_start(w_sb[name][:, :4, :], wr[:, :4, :])
                nc.sync.dma_start(w_sb[name][:, 4:, :], wr[:, 4:, :])

            # Hoist tile-0's x^T DMA ahead of the weight DMAs so the first
            # kd GEMM only waits for xt0 + k1 (~1.4 MB), not all weights.
            hoisted_xt0 = None
            if hw_repeats == 1:
                hoisted_xt0 = xt_p.tile([128, 24, 128], FP8, tag="xt",
                                        name="xt")
                nc.sync.dma_start(hoisted_xt0[:], xtp_d[0])
            wdma("k1")
            wdma("k2")
            wdma("q1")
            wdma("q2")
            nc.sync.dma_start(
                l1_sb[:], l1_d.rearrange("(ko p) n -> p ko n", p=128))
            nc.sync.dma_start(l2_rep[:], l2r_d[:])
            wdma("v")
            wdma("o")
            make_identity(nc, id_bf16[:])

            psB = psA if v["vride"] else psBpool

            # ---------------- phase A: projections + scores + dmats ---------
            def phase_A(mt, first=False):
                r0 = mt * 128
                if first and hoisted_xt0 is not None:
                    xt = hoisted_xt0
                else:
                    xt = xt_p.tile([128, 24, 128], FP8, tag="xt", name="xt")
                    nc.sync.dma_start(xt[:], xtp_d[mt])
                x_nat = xnat_p.tile([128, DIM], BF16, tag="xnat", name="xnat")
                nc.sync.dma_start(x_nat[:], x_d[r0:r0 + 128, :])

                # GEMM helper: K=1024 via 4 DR chunk-pairs into [128,512]
                # psum halves. Returns the two live psum tiles.
                # i (stationary x^T chunk) outer, n (moving half) inner so
                # consecutive MM pairs share one LDWEIGHTS.
                def gemm(kbase, wname, pool, tag):
                    halves = [pool.tile([128, 512], F32, tag=tag, name=tag)
                              for _ in range(2)]
                    if v["ldw_share"]:
                        for i in range(4):
                            ksl = slice(kbase + 2 * i, kbase + 2 * i + 2)
                            wsl = slice(2 * i, 2 * i + 2)
                            for n in range(2):
                                nsl = slice(n * 512, (n + 1) * 512)
                                nc.tensor.matmul(
                                    halves[n][:], xt[:, ksl, :],
                                    w_sb[wname][:, wsl, nsl],
                                    start=(i == 0), stop=(i == 3),
                                    perf_mode=DR)
                    else:
                        for n in range(2):
                            nsl = slice(n * 512, (n + 1) * 512)
                            for i in range(4):
                                ksl = slice(kbase + 2 * i, kbase + 2 * i + 2)
                                wsl = slice(2 * i, 2 * i + 2)
                                nc.tensor.matmul(
                                    halves[n][:], xt[:, ksl, :],
                                    w_sb[wname][:, wsl, nsl],
                                    start=(i == 0), stop=(i == 3),
                                    perf_mode=DR)
                    return halves

                # r_all column layout: si*8 + q*4 + h.
                r_all = small_p.tile([128, 16], F32, tag="rall", name="rall")
                scratch = scr_p.tile([128, 512], BF16, tag="scr", name="scr")

                def scores(si, q, halves):
                    """Fused DVE stt from the Q psum halves: never evicted."""
                    kd = kdiff[f"k{si + 1}"]
                    for h in range(H):
                        ps = halves[h // 2]
                        psl = slice((h % 2) * 256, (h % 2) * 256 + 256)
                        col = si * 8 + q * 4 + h
                        nc.vector.scalar_tensor_tensor(
                            scratch[:, (h % 2) * 256:(h % 2) * 256 + 256],
                            ps[:, psl], 1.0,
                            kd[:, h * 256:(h + 1) * 256],
                            AluOp.mult, AluOp.mult,
                            accum_out=r_all[:, col:col + 1])

                vs8 = (v_p.tile([128, 2, SD], FP8, tag="vs8", name="vs8")
                       if v["c8"] else None)

                def evict_v(tok, halves):
                    if v["c8"]:
                        vt = vs8.rearrange("b kv (f c) -> b kv f c", c=512)
                        for n, ps in enumerate(halves):
                            nc.scalar.mul(vt[:, tok, n, :], ps[:],
                                          X_SCALE / QK_SCALE)
                        return vs8
                    vt = v_p.tile([128, SD], BF16, tag=f"v_{tok}",
                                  name=f"v_{tok}")
                    for n, ps in enumerate(halves):
                        nc.scalar.mul(vt[:, n * 512:(n + 1) * 512], ps[:],
                                      1.0 / QK_SCALE)
                    return vt

                kdiff = {}
                vproj = {}
                if v["fuse_proj"]:
                    # kd-pass: kd1+kd2 interleaved — each xtd chunk stationary
                    # serves 4 MMs (one LDWEIGHTS per chunk).
                    kps = {k: [psA.tile([128, 512], F32, tag="pA", name="pA")
                               for _ in range(2)] for k in ("k1", "k2")}
                    for i in range(4):
                        ksl = slice(16 + 2 * i, 16 + 2 * i + 2)
                        wsl = slice(2 * i, 2 * i + 2)
                        for kname in ("k1", "k2"):
                            for n in range(2):
                                nsl = slice(n * 512, (n + 1) * 512)
                                nc.tensor.matmul(
                                    kps[kname][n][:], xt[:, ksl, :],
                                    w_sb[kname][:, wsl, nsl],
                                    start=(i == 0), stop=(i == 3),
                                    perf_mode=DR)
                    for kname in ("k1", "k2"):
                        kd = kd_p.tile([128, SD], BF16, tag=f"kd_{kname}",
                                       name=f"kd_{kname}")
                        kdiff[kname] = kd
                        for n, ps in enumerate(kps[kname]):
                            nc.scalar.copy(kd[:, n * 512:(n + 1) * 512],
                                           ps[:])

                    # q-pass: per token, q1+q2(+v)+lam ride each x^T chunk
                    # stationary (5 or 7 MMs per LDWEIGHTS).
                    names = ("q1", "q2", "v") if v["vride"] else ("q1", "q2")
                    ps_lam = psL.tile([128, LH], F32, tag="pL", name="pL")
                    for tok in range(2):
                        qps = {si: [psA.tile([128, 512], F32, tag="pA",
                                             name="pA") for _ in range(2)]
                               for si in range(len(names))}
                        for i in range(4):
                            ksl = slice(tok * 8 + 2 * i, tok * 8 + 2 * i + 2)
                            wsl = slice(2 * i, 2 * i + 2)
                            for si, qname in enumerate(names):
                                for n in range(2):
                                    nsl = slice(n * 512, (n + 1) * 512)
                                    nc.tensor.matmul(
                                        qps[si][n][:], xt[:, ksl, :],
                                        w_sb[qname][:, wsl, nsl],
                                        start=(i == 0), stop=(i == 3),
                                        perf_mode=DR)
                            nc.tensor.matmul(
                                ps_lam[:], xt[:, ksl, :], l1_sb[:, ksl, :],
                                start=(tok == 0 and i == 0),
                                stop=(tok == 1 and i == 3), perf_mode=DR,
                                skip_group_check=True)
                        for si in range(2):
                            scores(si, tok, qps[si])
                        if v["vride"]:
                            vproj[tok] = evict_v(tok, qps[2])
                else:
                    # Kdiff GEMMs (stationary = host-computed xtd chunks),
                    # evicted to bf16 on ACT.
                    for kname in ("k1", "k2"):
                        halves = gemm(16, kname, psA, "pA")
                        kd = kd_p.tile([128, SD], BF16, tag=f"kd_{kname}",
                                       name=f"kd_{kname}")
                        kdiff[kname] = kd
                        for n, ps in enumerate(halves):
                            nc.scalar.copy(kd[:, n * 512:(n + 1) * 512],
                                           ps[:])
                    for si, qname in enumerate(("q1", "q2")):
                        for q in range(2):
                            halves = gemm(q * 8, qname, psA, "pA")
                            scores(si, q, halves)
                    # lam MLP hidden: standalone GEMM over all 16 chunks.
                    ps_lam = psL.tile([128, LH], F32, tag="pL", name="pL")
                    for i in range(8):
                        nc.tensor.matmul(
                            ps_lam[:], xt[:, 2 * i:2 * i + 2, :],
                            l1_sb[:, 2 * i:2 * i + 2, :],
                            start=(i == 0), stop=(i == 7), perf_mode=DR)

                # V projections (evicted on ACT; under vride they were
                # computed inside the q-pass).  c8: fp8 16*V into a shared
                # kv-interleaved tile (DoubleRow stationary for the combine).
                for tok in ([] if v["vride"] else range(2)):
                    halves = gemm(tok * 8, "v", psA, "pA")
                    vproj[tok] = evict_v(tok, halves)

                # lambda = sigmoid(relu(H') . L2/1024): relu folds into the
                # logit stt as (psum max 0) * l2, with free accumulate.
                hscr = h_p.tile([128, LH], F32, tag="hs", name="hs")
                logit = small_p.tile([128, 1], F32, tag="logit", name="logit")
                nc.vector.scalar_tensor_tensor(
                    hscr[:], ps_lam[:], 0.0, l2_rep[:], AluOp.max, AluOp.mult,
                    accum_out=logit[:])
                lam = small_p.tile([128, 1], F32, tag="lam", name="lam")
                nc.scalar.activation(lam[:], logit[:], ActFn.Sigmoid)
                eng = nc.gpsimd if v["pool_ops"] else nc.vector
                if v["nodmat"]:
                    dmats = {(q, h, kv): id_bf16 for q in range(2)
                             for h in range(H) for kv in range(2)}
                    return dict(r0=r0, x_nat=x_nat, vproj=vproj, dmats=dmats)
                u = small_p.tile([128, 1], F32, tag="u", name="u")
                eng.tensor_scalar(u[:], lam[:], -1.0, 1.0,
                                  AluOp.mult, AluOp.add)

                # a = sigmoid(SCALE/2^20 * r) for all 16 scores at once (ACT)
                a_all = small_p.tile([128, 16], F32, tag="aall", name="aall")
                nc.scalar.activation(a_all[:], r_all[:], ActFn.Sigmoid,
                                     scale=float(SCALE / QK_SCALE ** 2))

                # diff-attn weights -> diag matrices (all built on Pool)
                dmats = {}
                for q in range(2):
                    a1q = a_all[:, q * 4:(q + 1) * 4]
                    a2q = a_all[:, 8 + q * 4:8 + (q + 1) * 4]
                    t = small_p.tile([128, H], F32, tag=f"t{q}", name=f"t{q}")
                    eng.tensor_scalar_mul(t[:], a2q, lam[:])
                    w0q = small_p.tile([128, H], F32, tag=f"w0{q}",
                                       name=f"w0{q}")
                    eng.tensor_tensor(w0q[:], a1q, t[:], AluOp.subtract)
                    eng.tensor_scalar_max(w0q[:], w0q[:], 0.0)
                    w1q = small_p.tile([128, H], F32, tag=f"w1{q}",
                                       name=f"w1{q}")
                    eng.tensor_tensor(w1q[:], t[:], a1q, AluOp.subtract)
                    eng.tensor_scalar(w1q[:], w1q[:], u[:], 0.0,
                                            AluOp.add, AluOp.max)
                    for h in range(H):
                        if v["c8"]:
                            d8 = d_p.tile([128, 2, 128], FP8, tag="dmat8",
                                          name="dmat8")
                            eng.tensor_scalar_mul(
                                d8[:, 0, :], id_bf16[:], w0q[:, h:h + 1])
                            eng.tensor_scalar_mul(
                                d8[:, 1, :], id_bf16[:], w1q[:, h:h + 1])
                            dmats[(q, h)] = d8
                            continue
                        d0 = d_p.tile([128, 128], BF16, tag="dmat",
                                      name="dmat")
                        eng.tensor_scalar_mul(
                            d0[:], id_bf16[:], w0q[:, h:h + 1])
                        d1 = d_p.tile([128, 128], BF16, tag="dmat",
                                      name="dmat")
                        eng.tensor_scalar_mul(
                            d1[:], id_bf16[:], w1q[:, h:h + 1])
                        dmats[(q, h, 0)] = d0
                        dmats[(q, h, 1)] = d1

                return dict(r0=r0, x_nat=x_nat, vproj=vproj, dmats=dmats)

            # ---------------- phase B: combine + WO + output ----------------
            def phase_B(st):
                r0, x_nat = st["r0"], st["x_nat"]
                vproj, dmats = st["vproj"], st["dmats"]
                # A_q^T via diag matmuls, 4 ftiles packed per psum bank.
                # (ft, kv) stationary V chunk serves both q's MMs back to
                # back (different dmats/psums) so LDWEIGHTS count halves.
                at = {q: at_p.tile([128, 8, 128], FP8, tag=f"at{q}",
                                   name=f"at{q}") for q in range(2)}
                btag = "pA" if v["vride"] else "pB"
                for half in range(2):
                    psq = {q: psB.tile([128, 512], F32, tag=btag, name=btag)
                           for q in range(2)}
                    for fi in range(4):
                        ft = half * 4 + fi
                        h = ft // 2
                        fsl = slice(ft * 128, (ft + 1) * 128)
                        if v["c8"]:
                            vstk = vproj[0].rearrange(
                                "b kv (f c) -> b kv f c", c=128)
                            for q in range(2):
                                nc.tensor.matmul(
                                    psq[q][:, fi * 128:(fi + 1) * 128],
                                    vstk[:, :, ft, :], dmats[(q, h)][:],
                                    start=True, stop=True, perf_mode=DR,
                                    skip_group_check=(fi != 0))
                            continue
                        for kv in range(2):
                            for q in range(2):
                                nc.tensor.matmul(
                                    psq[q][:, fi * 128:(fi + 1) * 128],
                                    vproj[kv][:, fsl], dmats[(q, h, kv)][:],
                                    start=(kv == 0), stop=(kv == 1),
                                    skip_group_check=(fi != 0))
                    at_scale = A_SCALE / X_SCALE if v["c8"] else A_SCALE
                    for q in range(2):
                        nc.scalar.mul(
                            at[q][:, half * 4:half * 4 + 4, :],
                            psq[q][:].rearrange("b (f c) -> b f c", c=128),
                            float(at_scale))

                # out_q = (A'_q @ WO')/512 + tokens_q   (fp8 DoubleRow);
                # i (stationary A^T chunk) outer, n inner: shared LDWEIGHTS.
                for q in range(2):
                    pso = [psB.tile([128, 512], F32, tag=btag, name=btag)
                           for _ in range(2)]
                    for i in range(4):
                        for n in range(2):
                            nsl = slice(n * 512, (n + 1) * 512)
                            nc.tensor.matmul(
                                pso[n][:], at[q][:, 2 * i:2 * i + 2, :],
                                w_sb["o"][:, 2 * i:2 * i + 2, nsl],
                                start=(i == 0), stop=(i == 3), perf_mode=DR)
                    for n in range(2):
                        o_sb = o_p.tile([128, 512], F32, tag="o", name="o")
                        osl = slice(q * SD + n * 512, q * SD + n * 512 + 512)
                        nc.vector.scalar_tensor_tensor(
                            o_sb[:], pso[n][:], 1.0 / O_SCALE, x_nat[:, osl],
                            AluOp.mult, AluOp.add)
                        nc.sync.dma_start(out_d[r0:r0 + 128, osl], o_sb[:])

            # ---------------- main loop (A/B software pipeline) -------------
            rep_cm = (tc.For_i(0, hw_repeats, 1) if hw_repeats > 1
                      else contextlib.nullcontext())
            with rep_cm:
                pending = None
                for mt_rep in range(n_mtiles * repeats):
                    mt = mt_rep % n_mtiles
                    st = phase_A(mt, first=(mt_rep == 0))
                    if v["nob"]:
                        continue
                    if not v["pipeline"]:
                        phase_B(st)
                        continue
                    if pending is not None:
                        phase_B(pending)
                    pending = st
                if pending is not None:
                    phase_B(pending)

    split_excess_waits(nc)
    return nc


_NC_CACHE = {}


def _get_nc(n_samples):
    if n_samples not in _NC_CACHE:
        _NC_CACHE[n_samples] = build_kernel(n_samples)
    return _NC_CACHE[n_samples]


def host_prep(inputs, n_samples=B_CORE):
    """Host-side shard + dtype/layout prep. Returns in_maps for 8 cores."""
    x = np.ascontiguousarray(np.asarray(inputs["x"], dtype=np.float32))
    assert x.shape[0] == N_CORES * n_samples and x.shape[1] == DIM
    f8 = ml_dtypes.float8_e4m3fn
    bf16 = ml_dtypes.bfloat16
    ws = {}
    for k in ("WQ1_w", "WK1_w", "WQ2_w", "WK2_w", "WV_w", "WO_w", "L1_w"):
        ws[k] = np.ascontiguousarray(
            (np.asarray(inputs[k], dtype=np.float32) * W_SCALE).astype(f8))
    l2rep = np.ascontiguousarray(
        np.broadcast_to(np.asarray(inputs["L2_w"], dtype=np.float32)
                        .reshape(1, LH) / QK_SCALE, (128, LH)))
    n_mtiles = n_samples // 128
    # scale+cast once (contiguous, fast), then one big strided transpose in
    # fp8: [core, mt, b, ft, p] -> [core, mt, p, ft, b]; then append the
    # host-computed token difference (T0-T1)^T as ftiles 16..23.
    xb = (x * X_SCALE).astype(f8).reshape(N_CORES, n_mtiles, 128, 16, 128)
    xt = xb.transpose(0, 1, 4, 3, 2)  # [core, mt, p, ft, b]
    xtd = (xt[:, :, :, 0:8, :].astype(np.float32)
           - xt[:, :, :, 8:16, :].astype(np.float32)).astype(f8)
    xtp_all = np.ascontiguousarray(
        np.concatenate([xt, xtd], axis=3))  # [core, mt, p, 24, b]
    xb16 = x.astype(bf16).reshape(N_CORES, n_samples, DIM)
    in_maps = []
    for c in range(N_CORES):
        m = {"xb": xb16[c], "xtp": xtp_all[c], "L2r": l2rep}
        m.update(ws)
        in_maps.append(m)
    return in_maps


def kernel(**inputs):
    from concourse.bass_utils import run_bass_kernel_spmd

    nc = _get_nc(B_CORE)
    in_maps = host_prep(inputs, B_CORE)
    res = run_bass_kernel_spmd(nc, in_maps, core_ids=list(range(N_CORES)))
    return np.concatenate([res.results[c]["out"] for c in range(N_CORES)],
                          axis=0)


# revision 29
# speedup vs baseline: 1.1163x; 1.1163x over previous
"""Trainium2 Bass kernel for AdaptiveDiffAttention.

Pure data parallel across 8 NeuronCores: each core processes B/8 = 2048
samples with a replicated copy of the weights. No collectives.

Math per sample b (seq len 2, heads 4, head dim 256):
  tokens  = x.reshape(2, 1024)
  lam     = sigmoid(relu(x @ L1) @ L2)
  Q,K,V   = tokens @ W*  (per token)
  softmax over 2 keys => a_q = sigmoid(SCALE * <Q_q, K_0 - K_1>)
  w0_q = relu(a1_q - lam*a2_q); w1_q = relu((1-a1_q) - lam*(1-a2_q))
  A_q  = w0_q * V_0 + w1_q * V_1   (per head)
  out_q = A_q @ WO + tokens_q

All big GEMMs run as fp8e4 DoubleRow matmuls. Scale management (TRN
e4m3 normal range [2^-6, 240]):
  x8 = 16*x, W' = 64*W  =>  Q'/Kd' psum = 1024*(Q/Kd)
  score sigmoid folds SCALE/2^20; V evicted with 1/1024 to natural bf16;
  A evicted with x8 to fp8 (stationary for the fp8 WO matmul);
  WO' = 64*WO => psum = 512*out_attn; o = psum/512 + x (bf16 residual).

Engine balance (v2): the v1 kernel was vector-engine-bound (~21us/tile
DVE busy vs ~23us PE). v2 rebalances so the PE is the only bottleneck:
  - x^T and the token difference (T0-T1)^T are precomputed on host and
    shipped fp8 (no on-chip transposes or subtracts).
  - Q projections are never evicted: the per-head score dot products run
    as fused DVE scalar_tensor_tensor (mult+mult+accum) reading the Q
    psum directly (GPSIMD/Pool cannot access PSUM, and its per-op
    dispatch overhead measured ~17us/tile — everything elementwise stays
    on DVE/ACT).
  - Kd/V/A^T psum evictions on ACT; the WO eviction + residual add are a
    single DVE scalar_tensor_tensor; lam's relu+dot folds into one DVE
    stt with accumulate ((psum max 0) * L2, accum).
  - x ships bf16 (residual precision is ample; halves that DMA).
  - The attention combine runs as fp8 DoubleRow diag-matmuls: V evicted
    as 16*V fp8 with V_0/V_1 kv-interleaved so one stationary covers
    both keys (16 MMs + 8 LDWEIGHTS instead of 32 + 32; costs ~+0.2e-2
    rel err, total ~1.25e-2 vs the 2e-2 gate).
  - LDWEIGHTS is expensive when exposed (~213ns per DoubleRow load, and
    walrus's ldw-opt is disabled and hard-fails when enabled), so every
    GEMM loop keeps the stationary operand outermost: kd1+kd2 share the
    xtd chunk stationaries, q1+q2+V+lam all ride each x^T chunk
    stationary (7 MMs per load), and WO shares each A^T chunk across
    both N halves.
Software pipelining: each tile is split into phase A (projection GEMMs,
scores, lam, dmats) and phase B (combine + WO + output).  Emission order
A(0) A(1) B(0) A(2) B(1) ... so the PE stream of tile t+1's projections
covers the DVE score->dmat chain of tile t; the PE never waits on the
post-GEMM pipeline (measured +10.6us/tile without it).  PSUM granularity
is [128,512] (one bank): a 7-buf ring shared by the projection passes
and phase B, plus 1 lam bank = 8 banks.

Startup: weight DMAs are emitted in first-use order (K1/K2 first, WO
last) and tile-0's x^T DMA is hoisted ahead of them, so the first
matmul waits on ~1.4 MB of transfers rather than all ~6.9 MB of weights.
"""

import contextlib
import sys

for _p in ("/opt/trn_rl_repo", "/root/.axon_site/_ro/trn_rl_repo"):
    if _p not in sys.path:
        sys.path.append(_p)

import numpy as np
import ml_dtypes

import concourse.bass as bass
import concourse.mybir as mybir
import bass_rust
from concourse.tile import TileContext
from concourse.masks import make_identity

F32 = mybir.dt.float32
BF16 = mybir.dt.bfloat16
FP8 = mybir.dt.float8e4

DIM = 2048
SD = 1024
H = 4
HD = 256
LH = 256
SCALE = HD ** -0.5
N_CORES = 8
B_FULL = 16384
B_CORE = B_FULL // N_CORES  # 2048

X_SCALE = 16.0
W_SCALE = 64.0
QK_SCALE = X_SCALE * W_SCALE          # 1024: Q'/Kd'/V' = 1024 * natural
A_SCALE = 8.0                          # at' = 8*A in fp8
O_SCALE = A_SCALE * W_SCALE            # 512: WO psum = 512 * out_attn

AluOp = mybir.AluOpType
ActFn = mybir.ActivationFunctionType
DR = mybir.MatmulPerfMode.DoubleRow


def split_excess_waits(nc, max_waits=1):
    """Walrus codegen in this container rejects >1 sync wait on CTRL-class
    instructions. Move excess waits onto chained nops before the offender."""
    for f in nc.m.functions:
        for bb in f.blocks:
            new_insts = []
            for inst in bb.instructions:
                si = inst.sync_info
                if si is not None and si.on_wait and len(si.on_wait) > max_waits:
                    waits = list(si.on_wait)
                    extra, keep = waits[:-max_waits], waits[-max_waits:]
                    for ci in range(0, len(extra), max_waits):
                        chunk = extra[ci:ci + max_waits]
                        nop = mybir.InstNoOp(name=f"{inst.name}-wsplit{ci}")
                        nop.engine = inst.engine
                        nop.sync_info = bass_rust.SyncInfo(
                            on_wait=chunk, on_update=[])
                        nc.register_instruction(nop, overwrite=True)
                        new_insts.append(nop)
                    inst.sync_info = bass_rust.SyncInfo(
                        on_wait=keep, on_update=list(si.on_update or []))
                new_insts.append(inst)
            bb.instructions = new_insts


DEFAULT_VARIANT = dict(
    pool_ops=False,     # dmats + small weight math on Pool (else DVE).
                        # False: GPSIMD per-op dispatch measured ~17us/tile.
    score_mode="stt",   # "stt": fused DVE stt from psum; "evict": v1-style
    pipeline=True,      # A/B software pipelining
    ldw_share=True,     # stationary-outer loop order in GEMMs
    fuse_proj=True,     # kd1+kd2 / q1+q2+lam share stationary x^T chunks
    nob=False,          # timing bisect: skip phase B entirely
    nodmat=False,       # timing bisect: combine reads identity, no dmat ops
    psl2=False,         # psA=4 + psL=2 (lam psum double buffered)
    vride=True,         # V MMs ride the q-pass stationaries (7 MM / LDW);
                        # single shared 7-buf psum ring for A and B
    c8=True,            # combine via fp8 DoubleRow: V0/V1 stacked stationary
                        # (16*V fp8), stacked diag weights; 16 MMs / 8 LDW
    bmix=False,         # interleave combine(q1) MMs into wo(q0) stream so
                        # combine LDWEIGHTS hide under 512-col wo streams
)


def build_kernel(n_samples=B_CORE, repeats=1, hw_repeats=1, variant=None):
    """Build the single-core Bass graph. n_samples must be a multiple of 128.

    hw_repeats: hardware For_i loop around the whole tile loop (graph does
    not grow) — used for timing with large in-NEFF repeat factors."""
    v = dict(DEFAULT_VARIANT)
    if variant:
        v.update(variant)
    assert n_samples % 128 == 0
    n_mtiles = n_samples // 128

    nc = bass.Bass()

    # x in bf16 (residual only); x^T per m-tile in fp8 (16*x):
    # [mt, feat_in_tile(p), ftile, b] with ftile 0..15 = x^T, 16..23 = xtd^T
    x_d = nc.declare_dram_parameter("xb", [n_samples, DIM], BF16,
                                    isOutput=False)
    xtp_d = nc.declare_dram_parameter(
        "xtp", [n_mtiles, 128, 24, 128], FP8, isOutput=False)
    w_d = {}
    for name, pname in (("q1", "WQ1_w"), ("k1", "WK1_w"), ("q2", "WQ2_w"),
                        ("k2", "WK2_w"), ("v", "WV_w"), ("o", "WO_w")):
        w_d[name] = nc.declare_dram_parameter(pname, [SD, SD], FP8,
                                              isOutput=False)
    l1_d = nc.declare_dram_parameter("L1_w", [DIM, LH], FP8, isOutput=False)
    l2r_d = nc.declare_dram_parameter("L2r", [128, LH], F32, isOutput=False)
    out_d = nc.declare_dram_parameter("out", [n_samples, DIM], F32,
                                      isOutput=True)

    with TileContext(nc) as tc:
        with (
            tc.tile_pool(name="const", bufs=1) as const,
            tc.tile_pool(name="xnat", bufs=2) as xnat_p,
            tc.tile_pool(name="xt", bufs=2) as xt_p,
            tc.tile_pool(name="kdp", bufs=2) as kd_p,
            tc.tile_pool(name="vbuf", bufs=2) as v_p,
            tc.tile_pool(name="scr", bufs=2) as scr_p,
            tc.tile_pool(name="small", bufs=2) as small_p,
            tc.tile_pool(name="hbuf", bufs=2) as h_p,
            tc.tile_pool(name="dpool", bufs=32) as d_p,
            tc.tile_pool(name="at", bufs=2) as at_p,
            tc.tile_pool(name="obuf", bufs=8) as o_p,
            tc.tile_pool(name="psA", bufs=(6 if v["psl2"] else 7)
                         if v["vride"] else (4 if v["psl2"] else 5),
                         space="PSUM") as psA,
            tc.tile_pool(name="psL", bufs=2 if v["psl2"] else 1,
                         space="PSUM") as psL,
            tc.tile_pool(name="psB", bufs=2, space="PSUM") as psBpool,
        ):
            # ---------------- resident weights (already fp8, x64) -----------
            w_sb = {}
            for name in ("k1", "k2", "q1", "q2", "v", "o"):
                wt = const.tile([128, 8, SD], FP8, name=f"w_{name}")
                w_sb[name] = wt
            l1_sb = const.tile([128, 16, LH], FP8, name="l1")
            l2_rep = const.tile([128, LH], F32, name="l2rep")
            id_bf16 = const.tile([128, 128], BF16, name="id16")

            def wdma(name):
                wr = w_d[name].rearrange("(ko p) n -> p ko n", p=128)
                nc.sync.dma_start(w_sb[name][:, :4, :], wr[:, :4, :])
                nc.sync.dma_start(w_sb[name][:, 4:, :], wr[:, 4:, :])

            # Hoist tile-0's x^T DMA ahead of the weight DMAs so the first
            # kd GEMM only waits for xt0 + k1 (~1.4 MB), not all weights.
            hoisted_xt0 = None
            if hw_repeats == 1:
                hoisted_xt0 = xt_p.tile([128, 24, 128], FP8, tag="xt",
                                        name="xt")
                nc.sync.dma_start(hoisted_xt0[:], xtp_d[0])
            wdma("k1")
            wdma("k2")
            wdma("q1")
            wdma("q2")
            nc.sync.dma_start(
                l1_sb[:], l1_d.rearrange("(ko p) n -> p ko n", p=128))
            nc.sync.dma_start(l2_rep[:], l2r_d[:])
            wdma("v")
            wdma("o")
            make_identity(nc, id_bf16[:])

            psB = psA if v["vride"] else psBpool

            # ---------------- phase A: projections + scores + dmats ---------
            def phase_A(mt, first=False):
                r0 = mt * 128
                if first and hoisted_xt0 is not None:
                    xt = hoisted_xt0
                else:
                    xt = xt_p.tile([128, 24, 128], FP8, tag="xt", name="xt")
                    nc.sync.dma_start(xt[:], xtp_d[mt])
                x_nat = xnat_p.tile([128, DIM], BF16, tag="xnat", name="xnat")
                nc.sync.dma_start(x_nat[:], x_d[r0:r0 + 128, :])

                # GEMM helper: K=1024 via 4 DR chunk-pairs into [128,512]
                # psum halves. Returns the two live psum tiles.
                # i (stationary x^T chunk) outer, n (moving half) inner so
                # consecutive MM pairs share one LDWEIGHTS.
                def gemm(kbase, wname, pool, tag):
                    halves = [pool.tile([128, 512], F32, tag=tag, name=tag)
                              for _ in range(2)]
                    if v["ldw_share"]:
                        for i in range(4):
                            ksl = slice(kbase + 2 * i, kbase + 2 * i + 2)
                            wsl = slice(2 * i, 2 * i + 2)
                            for n in range(2):
                                nsl = slice(n * 512, (n + 1) * 512)
                                nc.tensor.matmul(
                                    halves[n][:], xt[:, ksl, :],
                                    w_sb[wname][:, wsl, nsl],
                                    start=(i == 0), stop=(i == 3),
                                    perf_mode=DR)
                    else:
                        for n in range(2):
                            nsl = slice(n * 512, (n + 1) * 512)
                            for i in range(4):
                                ksl = slice(kbase + 2 * i, kbase + 2 * i + 2)
                                wsl = slice(2 * i, 2 * i + 2)
                                nc.tensor.matmul(
                                    halves[n][:], xt[:, ksl, :],
                                    w_sb[wname][:, wsl, nsl],
                                    start=(i == 0), stop=(i == 3),
                                    perf_mode=DR)
                    return halves

                # r_all column layout: si*8 + q*4 + h.
                r_all = small_p.tile([128, 16], F32, tag="rall", name="rall")
                scratch = scr_p.tile([128, 512], BF16, tag="scr", name="scr")

                def scores(si, q, halves):
                    """Fused DVE stt from the Q psum halves: never evicted."""
                    kd = kdiff[f"k{si + 1}"]
                    for h in range(H):
                        ps = halves[h // 2]
                        psl = slice((h % 2) * 256, (h % 2) * 256 + 256)
                        col = si * 8 + q * 4 + h
                        nc.vector.scalar_tensor_tensor(
                            scratch[:, (h % 2) * 256:(h % 2) * 256 + 256],
                            ps[:, psl], 1.0,
                            kd[:, h * 256:(h + 1) * 256],
                            AluOp.mult, AluOp.mult,
                            accum_out=r_all[:, col:col + 1])

                vs8 = (v_p.tile([128, 2, SD], FP8, tag="vs8", name="vs8")
                       if v["c8"] else None)

                def evict_v(tok, halves):
                    if v["c8"]:
                        vt = vs8.rearrange("b kv (f c) -> b kv f c", c=512)
                        for n, ps in enumerate(halves):
                            nc.scalar.mul(vt[:, tok, n, :], ps[:],
                                          X_SCALE / QK_SCALE)
                        return vs8
                    vt = v_p.tile([128, SD], BF16, tag=f"v_{tok}",
                                  name=f"v_{tok}")
                    for n, ps in enumerate(halves):
                        nc.scalar.mul(vt[:, n * 512:(n + 1) * 512], ps[:],
                                      1.0 / QK_SCALE)
                    return vt

                kdiff = {}
                vproj = {}
                if v["fuse_proj"]:
                    # kd-pass: kd1+kd2 interleaved — each xtd chunk stationary
                    # serves 4 MMs (one LDWEIGHTS per chunk).
                    kps = {k: [psA.tile([128, 512], F32, tag="pA", name="pA")
                               for _ in range(2)] for k in ("k1", "k2")}
                    for i in range(4):
                        ksl = slice(16 + 2 * i, 16 + 2 * i + 2)
                        wsl = slice(2 * i, 2 * i + 2)
                        for kname in ("k1", "k2"):
                            for n in range(2):
                                nsl = slice(n * 512, (n + 1) * 512)
                                nc.tensor.matmul(
                                    kps[kname][n][:], xt[:, ksl, :],
                                    w_sb[kname][:, wsl, nsl],
                                    start=(i == 0), stop=(i == 3),
                                    perf_mode=DR)
                    for kname in ("k1", "k2"):
                        kd = kd_p.tile([128, SD], BF16, tag=f"kd_{kname}",
                                       name=f"kd_{kname}")
                        kdiff[kname] = kd
                        for n, ps in enumerate(kps[kname]):
                            nc.scalar.copy(kd[:, n * 512:(n + 1) * 512],
                                           ps[:])

                    # q-pass: per token, q1+q2(+v)+lam ride each x^T chunk
                    # stationary (5 or 7 MMs per LDWEIGHTS).
                    names = ("q1", "q2", "v") if v["vride"] else ("q1", "q2")
                    ps_lam = psL.tile([128, LH], F32, tag="pL", name="pL")
                    for tok in range(2):
                        qps = {si: [psA.tile([128, 512], F32, tag="pA",
                                             name="pA") for _ in range(2)]
                               for si in range(len(names))}
                        for i in range(4):
                            ksl = slice(tok * 8 + 2 * i, tok * 8 + 2 * i + 2)
                            wsl = slice(2 * i, 2 * i + 2)
                            for si, qname in enumerate(names):
                                for n in range(2):
                                    nsl = slice(n * 512, (n + 1) * 512)
                                    nc.tensor.matmul(
                                        qps[si][n][:], xt[:, ksl, :],
                                        w_sb[qname][:, wsl, nsl],
                                        start=(i == 0), stop=(i == 3),
                                        perf_mode=DR)
                            nc.tensor.matmul(
                                ps_lam[:], xt[:, ksl, :], l1_sb[:, ksl, :],
                                start=(tok == 0 and i == 0),
                                stop=(tok == 1 and i == 3), perf_mode=DR,
                                skip_group_check=True)
                        for si in range(2):
                            scores(si, tok, qps[si])
                        if v["vride"]:
                            vproj[tok] = evict_v(tok, qps[2])
                else:
                    # Kdiff GEMMs (stationary = host-computed xtd chunks),
                    # evicted to bf16 on ACT.
                    for kname in ("k1", "k2"):
                        halves = gemm(16, kname, psA, "pA")
                        kd = kd_p.tile([128, SD], BF16, tag=f"kd_{kname}",
                                       name=f"kd_{kname}")
                        kdiff[kname] = kd
                        for n, ps in enumerate(halves):
                            nc.scalar.copy(kd[:, n * 512:(n + 1) * 512],
                                           ps[:])
                    for si, qname in enumerate(("q1", "q2")):
                        for q in range(2):
                            halves = gemm(q * 8, qname, psA, "pA")
                            scores(si, q, halves)
                    # lam MLP hidden: standalone GEMM over all 16 chunks.
                    ps_lam = psL.tile([128, LH], F32, tag="pL", name="pL")
                    for i in range(8):
                        nc.tensor.matmul(
                            ps_lam[:], xt[:, 2 * i:2 * i + 2, :],
                            l1_sb[:, 2 * i:2 * i + 2, :],
                            start=(i == 0), stop=(i == 7), perf_mode=DR)

                # V projections (evicted on ACT; under vride they were
                # computed inside the q-pass).  c8: fp8 16*V into a shared
                # kv-interleaved tile (DoubleRow stationary for the combine).
                for tok in ([] if v["vride"] else range(2)):
                    halves = gemm(tok * 8, "v", psA, "pA")
                    vproj[tok] = evict_v(tok, halves)

                # lambda = sigmoid(relu(H') . L2/1024): relu folds into the
                # logit stt as (psum max 0) * l2, with free accumulate.
                hscr = h_p.tile([128, LH], F32, tag="hs", name="hs")
                logit = small_p.tile([128, 1], F32, tag="logit", name="logit")
                nc.vector.scalar_tensor_tensor(
                    hscr[:], ps_lam[:], 0.0, l2_rep[:], AluOp.max, AluOp.mult,
                    accum_out=logit[:])
                lam = small_p.tile([128, 1], F32, tag="lam", name="lam")
                nc.scalar.activation(lam[:], logit[:], ActFn.Sigmoid)
                eng = nc.gpsimd if v["pool_ops"] else nc.vector
                if v["nodmat"]:
                    dmats = {(q, h, kv): id_bf16 for q in range(2)
                             for h in range(H) for kv in range(2)}
                    return dict(r0=r0, x_nat=x_nat, vproj=vproj, dmats=dmats)
                u = small_p.tile([128, 1], F32, tag="u", name="u")
                eng.tensor_scalar(u[:], lam[:], -1.0, 1.0,
                                  AluOp.mult, AluOp.add)

                # a = sigmoid(SCALE/2^20 * r) for all 16 scores at once (ACT)
                a_all = small_p.tile([128, 16], F32, tag="aall", name="aall")
                nc.scalar.activation(a_all[:], r_all[:], ActFn.Sigmoid,
                                     scale=float(SCALE / QK_SCALE ** 2))

                # diff-attn weights -> diag matrices (all built on Pool)
                dmats = {}
                for q in range(2):
                    a1q = a_all[:, q * 4:(q + 1) * 4]
                    a2q = a_all[:, 8 + q * 4:8 + (q + 1) * 4]
                    t = small_p.tile([128, H], F32, tag=f"t{q}", name=f"t{q}")
                    eng.tensor_scalar_mul(t[:], a2q, lam[:])
                    w0q = small_p.tile([128, H], F32, tag=f"w0{q}",
                                       name=f"w0{q}")
                    eng.tensor_tensor(w0q[:], a1q, t[:], AluOp.subtract)
                    eng.tensor_scalar_max(w0q[:], w0q[:], 0.0)
                    w1q = small_p.tile([128, H], F32, tag=f"w1{q}",
                                       name=f"w1{q}")
                    eng.tensor_tensor(w1q[:], t[:], a1q, AluOp.subtract)
                    eng.tensor_scalar(w1q[:], w1q[:], u[:], 0.0,
                                            AluOp.add, AluOp.max)
                    for h in range(H):
                        if v["c8"]:
                            d8 = d_p.tile([128, 2, 128], FP8, tag="dmat8",
                                          name="dmat8")
                            eng.tensor_scalar_mul(
                                d8[:, 0, :], id_bf16[:], w0q[:, h:h + 1])
                            eng.tensor_scalar_mul(
                                d8[:, 1, :], id_bf16[:], w1q[:, h:h + 1])
                            dmats[(q, h)] = d8
                            continue
                        d0 = d_p.tile([128, 128], BF16, tag="dmat",
                                      name="dmat")
                        eng.tensor_scalar_mul(
                            d0[:], id_bf16[:], w0q[:, h:h + 1])
                        d1 = d_p.tile([128, 128], BF16, tag="dmat",
                                      name="dmat")
                        eng.tensor_scalar_mul(
                            d1[:], id_bf16[:], w1q[:, h:h + 1])
                        dmats[(q, h, 0)] = d0
                        dmats[(q, h, 1)] = d1

                return dict(r0=r0, x_nat=x_nat, vproj=vproj, dmats=dmats)

            # ---------------- phase B: combine + WO + output ----------------
            def phase_B(st):
                r0, x_nat = st["r0"], st["x_nat"]
                vproj, dmats = st["vproj"], st["dmats"]
                # A_q^T via diag matmuls, 4 ftiles packed per psum bank.
                # (ft, kv) stationary V chunk serves both q's MMs back to
                # back (different dmats/psums) so LDWEIGHTS count halves.
                at = {q: at_p.tile([128, 8, 128], FP8, tag=f"at{q}",
                                   name=f"at{q}") for q in range(2)}
                btag = "pA" if v["vride"] else "pB"
                for half in range(2):
                    psq = {q: psB.tile([128, 512], F32, tag=btag, name=btag)
                           for q in range(2)}
                    for fi in range(4):
                        ft = half * 4 + fi
                        h = ft // 2
                        fsl = slice(ft * 128, (ft + 1) * 128)
                        if v["c8"]:
                            vstk = vproj[0].rearrange(
                                "b kv (f c) -> b kv f c", c=128)
                            for q in range(2):
                                nc.tensor.matmul(
                                    psq[q][:, fi * 128:(fi + 1) * 128],
                                    vstk[:, :, ft, :], dmats[(q, h)][:],
                                    start=True, stop=True, perf_mode=DR,
                                    skip_group_check=(fi != 0))
                            continue
                        for kv in range(2):
                            for q in range(2):
                                nc.tensor.matmul(
                                    psq[q][:, fi * 128:(fi + 1) * 128],
                                    vproj[kv][:, fsl], dmats[(q, h, kv)][:],
                                    start=(kv == 0), stop=(kv == 1),
                                    skip_group_check=(fi != 0))
                    at_scale = A_SCALE / X_SCALE if v["c8"] else A_SCALE
                    for q in range(2):
                        nc.scalar.mul(
                            at[q][:, half * 4:half * 4 + 4, :],
                            psq[q][:].rearrange("b (f c) -> b f c", c=128),
                            float(at_scale))

                # out_q = (A'_q @ WO')/512 + tokens_q   (fp8 DoubleRow);
                # i (stationary A^T chunk) outer, n inner: shared LDWEIGHTS.
                for q in range(2):
                    pso = [psB.tile([128, 512], F32, tag=btag, name=btag)
                           for _ in range(2)]
                    for i in range(4):
                        for n in range(2):
                            nsl = slice(n * 512, (n + 1) * 512)
                            nc.tensor.matmul(
                                pso[n][:], at[q][:, 2 * i:2 * i + 2, :],
                                w_sb["o"][:, 2 * i:2 * i + 2, nsl],
                                start=(i == 0), stop=(i == 3), perf_mode=DR)
                    for n in range(2):
                        o_sb = o_p.tile([128, 512], F32, tag="o", name="o")
                        osl = slice(q * SD + n * 512, q * SD + n * 512 + 512)
                        nc.vector.scalar_tensor_tensor(
                            o_sb[:], pso[n][:], 1.0 / O_SCALE, x_nat[:, osl],
                            AluOp.mult, AluOp.add)
                        nc.sync.dma_start(out_d[r0:r0 + 128, osl], o_sb[:])

            # ---------------- main loop (A/B software pipeline) -------------
            rep_cm = (tc.For_i(0, hw_repeats, 1) if hw_repeats > 1
                      else contextlib.nullcontext())
            with rep_cm:
                pending = None
                for mt_rep in range(n_mtiles * repeats):
                    mt = mt_rep % n_mtiles
                    st = phase_A(mt, first=(mt_rep == 0))
                    if v["nob"]:
                        continue
                    if not v["pipeline"]:
                        phase_B(st)
                        continue
                    if pending is not None:
                        phase_B(pending)
                    pending = st
                if pending is not None:
                    phase_B(pending)

    split_excess_waits(nc)
    return nc


_NC_CACHE = {}


def _get_nc(n_samples):
    if n_samples not in _NC_CACHE:
        _NC_CACHE[n_samples] = build_kernel(n_samples)
    return _NC_CACHE[n_samples]


def host_prep(inputs, n_samples=B_CORE):
    """Host-side shard + dtype/layout prep. Returns in_maps for 8 cores."""
    x = np.ascontiguousarray(np.asarray(inputs["x"], dtype=np.float32))
    assert x.shape[0] == N_CORES * n_samples and x.shape[1] == DIM
    f8 = ml_dtypes.float8_e4m3fn
    bf16 = ml_dtypes.bfloat16
    ws = {}
    for k in ("WQ1_w", "WK1_w", "WQ2_w", "WK2_w", "WV_w", "WO_w", "L1_w"):
        ws[k] = np.ascontiguousarray(
            (np.asarray(inputs[k], dtype=np.float32) * W_SCALE).astype(f8))
    l2rep = np.ascontiguousarray(
        np.broadcast_to(np.asarray(inputs["L2_w"], dtype=np.float32)
                        .reshape(1, LH) / QK_SCALE, (128, LH)))
    n_mtiles = n_samples // 128
    # scale+cast once (contiguous, fast), then one big strided transpose in
    # fp8: [core, mt, b, ft, p] -> [core, mt, p, ft, b]; then append the
    # host-computed token difference (T0-T1)^T as ftiles 16..23.
    xb = (x * X_SCALE).astype(f8).reshape(N_CORES, n_mtiles, 128, 16, 128)
    xt = xb.transpose(0, 1, 4, 3, 2)  # [core, mt, p, ft, b]
    xtd = (xt[:, :, :, 0:8, :].astype(np.float32)
           - xt[:, :, :, 8:16, :].astype(np.float32)).astype(f8)
    xtp_all = np.ascontiguousarray(
        np.concatenate([xt, xtd], axis=3))  # [core, mt, p, 24, b]
    xb16 = x.astype(bf16).reshape(N_CORES, n_samples, DIM)
    in_maps = []
    for c in range(N_CORES):
        m = {"xb": xb16[c], "xtp": xtp_all[c], "L2r": l2rep}
        m.update(ws)
        in_maps.append(m)
    return in_maps


def kernel(**inputs):
    from concourse.bass_utils import run_bass_kernel_spmd

    nc = _get_nc(B_CORE)
    in_maps = host_prep(inputs, B_CORE)
    res = run_bass_kernel_spmd(nc, in_maps, core_ids=list(range(N_CORES)))
    return np.concatenate([res.results[c]["out"] for c in range(N_CORES)],
                          axis=0)


# revision 31
# speedup vs baseline: 1.1457x; 1.0263x over previous
"""Trainium2 Bass kernel for AdaptiveDiffAttention.

Pure data parallel across 8 NeuronCores: each core processes B/8 = 2048
samples with a replicated copy of the weights. No collectives.

Math per sample b (seq len 2, heads 4, head dim 256):
  tokens  = x.reshape(2, 1024)
  lam     = sigmoid(relu(x @ L1) @ L2)
  Q,K,V   = tokens @ W*  (per token)
  softmax over 2 keys => a_q = sigmoid(SCALE * <Q_q, K_0 - K_1>)
  w0_q = relu(a1_q - lam*a2_q); w1_q = relu((1-a1_q) - lam*(1-a2_q))
  A_q  = w0_q * V_0 + w1_q * V_1   (per head)
  out_q = A_q @ WO + tokens_q

All big GEMMs run as fp8e4 DoubleRow matmuls. Scale management (TRN
e4m3 normal range [2^-6, 240]):
  x8 = 16*x, W' = 64*W  =>  Q'/Kd' psum = 1024*(Q/Kd)
  score sigmoid folds SCALE/2^20; V evicted with 1/1024 to natural bf16;
  A evicted with x8 to fp8 (stationary for the fp8 WO matmul);
  WO' = 64*WO => psum = 512*out_attn; o = psum/512 + x (bf16 residual).

Engine balance (v2): the v1 kernel was vector-engine-bound (~21us/tile
DVE busy vs ~23us PE). v2 rebalances so the PE is the only bottleneck:
  - x^T and the token difference (T0-T1)^T are precomputed on host and
    shipped fp8 (no on-chip transposes or subtracts).
  - Q projections are never evicted: the per-head score dot products run
    as fused DVE scalar_tensor_tensor (mult+mult+accum) reading the Q
    psum directly (GPSIMD/Pool cannot access PSUM, and its per-op
    dispatch overhead measured ~17us/tile — everything elementwise stays
    on DVE/ACT).
  - Kd/V/A^T psum evictions on ACT; the WO eviction + residual add are a
    single DVE scalar_tensor_tensor; lam's relu+dot folds into one DVE
    stt with accumulate ((psum max 0) * L2, accum).
  - x ships bf16 (residual precision is ample; halves that DMA).
  - The attention combine runs as fp8 DoubleRow diag-matmuls: V evicted
    as 16*V fp8 with V_0/V_1 kv-interleaved so one stationary covers
    both keys (16 MMs + 8 LDWEIGHTS instead of 32 + 32; costs ~+0.2e-2
    rel err, total ~1.25e-2 vs the 2e-2 gate).
  - LDWEIGHTS is expensive when exposed (~213ns per DoubleRow load, and
    walrus's ldw-opt is disabled and hard-fails when enabled), so every
    GEMM loop keeps the stationary operand outermost: kd1+kd2 share the
    xtd chunk stationaries, q1+q2+V+lam all ride each x^T chunk
    stationary (7 MMs per load), and WO shares each A^T chunk across
    both N halves.
Software pipelining: each tile is split into phase A (projection GEMMs,
scores, lam, dmats) and phase B (combine + WO + output).  Emission order
A(0) A(1) B(0) A(2) B(1) ... so the PE stream of tile t+1's projections
covers the DVE score->dmat chain of tile t; the PE never waits on the
post-GEMM pipeline (measured +10.6us/tile without it).  The combine MMs
of tile t-1 are additionally interleaved 1:1 between the kd-pass MMs of
tile t (cmix) so each combine LDWEIGHTS loads during a 242ns kd stream
instead of back-to-back with its own 60ns streams.  PSUM granularity
is [128,512] (one bank): a 7-buf ring shared by the projection passes
and phase B, plus 1 lam bank = 8 banks.

Startup: weight DMAs are emitted in first-use order (K1/K2 first, WO
last) and tile-0's x^T DMA is hoisted ahead of them, so the first
matmul waits on ~1.4 MB of transfers rather than all ~6.9 MB of weights.
"""

import contextlib
import sys

for _p in ("/opt/trn_rl_repo", "/root/.axon_site/_ro/trn_rl_repo"):
    if _p not in sys.path:
        sys.path.append(_p)

import numpy as np
import ml_dtypes

import concourse.bass as bass
import concourse.mybir as mybir
import bass_rust
from concourse.tile import TileContext
from concourse.masks import make_identity

F32 = mybir.dt.float32
BF16 = mybir.dt.bfloat16
FP8 = mybir.dt.float8e4

DIM = 2048
SD = 1024
H = 4
HD = 256
LH = 256
SCALE = HD ** -0.5
N_CORES = 8
B_FULL = 16384
B_CORE = B_FULL // N_CORES  # 2048

X_SCALE = 16.0
W_SCALE = 64.0
QK_SCALE = X_SCALE * W_SCALE          # 1024: Q'/Kd'/V' = 1024 * natural
A_SCALE = 8.0                          # at' = 8*A in fp8
O_SCALE = A_SCALE * W_SCALE            # 512: WO psum = 512 * out_attn

AluOp = mybir.AluOpType
ActFn = mybir.ActivationFunctionType
DR = mybir.MatmulPerfMode.DoubleRow


def split_excess_waits(nc, max_waits=1):
    """Walrus codegen in this container rejects >1 sync wait on CTRL-class
    instructions. Move excess waits onto chained nops before the offender."""
    for f in nc.m.functions:
        for bb in f.blocks:
            new_insts = []
            for inst in bb.instructions:
                si = inst.sync_info
                if si is not None and si.on_wait and len(si.on_wait) > max_waits:
                    waits = list(si.on_wait)
                    extra, keep = waits[:-max_waits], waits[-max_waits:]
                    for ci in range(0, len(extra), max_waits):
                        chunk = extra[ci:ci + max_waits]
                        nop = mybir.InstNoOp(name=f"{inst.name}-wsplit{ci}")
                        nop.engine = inst.engine
                        nop.sync_info = bass_rust.SyncInfo(
                            on_wait=chunk, on_update=[])
                        nc.register_instruction(nop, overwrite=True)
                        new_insts.append(nop)
                    inst.sync_info = bass_rust.SyncInfo(
                        on_wait=keep, on_update=list(si.on_update or []))
                new_insts.append(inst)
            bb.instructions = new_insts


DEFAULT_VARIANT = dict(
    pool_ops=False,     # dmats + small weight math on Pool (else DVE).
                        # False: GPSIMD per-op dispatch measured ~17us/tile.
    score_mode="stt",   # "stt": fused DVE stt from psum; "evict": v1-style
    pipeline=True,      # A/B software pipelining
    ldw_share=True,     # stationary-outer loop order in GEMMs
    fuse_proj=True,     # kd1+kd2 / q1+q2+lam share stationary x^T chunks
    nob=False,          # timing bisect: skip phase B entirely
    nodmat=False,       # timing bisect: combine reads identity, no dmat ops
    psl2=False,         # psA=4 + psL=2 (lam psum double buffered)
    vride=True,         # V MMs ride the q-pass stationaries (7 MM / LDW);
                        # single shared 7-buf psum ring for A and B
    c8=True,            # combine via fp8 DoubleRow: V0/V1 stacked stationary
                        # (16*V fp8), stacked diag weights; 16 MMs / 8 LDW
    bmix=False,         # (unused)
    cmix=True,          # emit combine(t-1) MMs 1:1 between kd-pass(t) MMs so
                        # each combine LDWEIGHTS hides under a 242ns kd stream
)


def build_kernel(n_samples=B_CORE, repeats=1, hw_repeats=1, variant=None):
    """Build the single-core Bass graph. n_samples must be a multiple of 128.

    hw_repeats: hardware For_i loop around the whole tile loop (graph does
    not grow) — used for timing with large in-NEFF repeat factors."""
    v = dict(DEFAULT_VARIANT)
    if variant:
        v.update(variant)
    assert n_samples % 128 == 0
    n_mtiles = n_samples // 128

    nc = bass.Bass()

    # x in bf16 (residual only); x^T per m-tile in fp8 (16*x):
    # [mt, feat_in_tile(p), ftile, b] with ftile 0..15 = x^T, 16..23 = xtd^T
    x_d = nc.declare_dram_parameter("xb", [n_samples, DIM], BF16,
                                    isOutput=False)
    xtp_d = nc.declare_dram_parameter(
        "xtp", [n_mtiles, 128, 24, 128], FP8, isOutput=False)
    w_d = {}
    for name, pname in (("q1", "WQ1_w"), ("k1", "WK1_w"), ("q2", "WQ2_w"),
                        ("k2", "WK2_w"), ("v", "WV_w"), ("o", "WO_w")):
        w_d[name] = nc.declare_dram_parameter(pname, [SD, SD], FP8,
                                              isOutput=False)
    l1_d = nc.declare_dram_parameter("L1_w", [DIM, LH], FP8, isOutput=False)
    l2r_d = nc.declare_dram_parameter("L2r", [128, LH], F32, isOutput=False)
    out_d = nc.declare_dram_parameter("out", [n_samples, DIM], F32,
                                      isOutput=True)

    with TileContext(nc) as tc:
        with (
            tc.tile_pool(name="const", bufs=1) as const,
            tc.tile_pool(name="xnat", bufs=2) as xnat_p,
            tc.tile_pool(name="xt", bufs=2) as xt_p,
            tc.tile_pool(name="kdp", bufs=2) as kd_p,
            tc.tile_pool(name="vbuf", bufs=2) as v_p,
            tc.tile_pool(name="scr", bufs=2) as scr_p,
            tc.tile_pool(name="small", bufs=2) as small_p,
            tc.tile_pool(name="hbuf", bufs=2) as h_p,
            tc.tile_pool(name="dpool", bufs=32) as d_p,
            tc.tile_pool(name="at", bufs=2) as at_p,
            tc.tile_pool(name="obuf", bufs=8) as o_p,
            tc.tile_pool(name="psA", bufs=(6 if v["psl2"] else 7)
                         if v["vride"] else (4 if v["psl2"] else 5),
                         space="PSUM") as psA,
            tc.tile_pool(name="psL", bufs=2 if v["psl2"] else 1,
                         space="PSUM") as psL,
            tc.tile_pool(name="psB", bufs=2, space="PSUM") as psBpool,
        ):
            # ---------------- resident weights (already fp8, x64) -----------
            w_sb = {}
            for name in ("k1", "k2", "q1", "q2", "v", "o"):
                wt = const.tile([128, 8, SD], FP8, name=f"w_{name}")
                w_sb[name] = wt
            l1_sb = const.tile([128, 16, LH], FP8, name="l1")
            l2_rep = const.tile([128, LH], F32, name="l2rep")
            id_bf16 = const.tile([128, 128], BF16, name="id16")

            def wdma(name):
                wr = w_d[name].rearrange("(ko p) n -> p ko n", p=128)
                nc.sync.dma_start(w_sb[name][:, :4, :], wr[:, :4, :])
                nc.sync.dma_start(w_sb[name][:, 4:, :], wr[:, 4:, :])

            # Hoist tile-0's x^T DMA ahead of the weight DMAs so the first
            # kd GEMM only waits for xt0 + k1 (~1.4 MB), not all weights.
            hoisted_xt0 = None
            if hw_repeats == 1:
                hoisted_xt0 = xt_p.tile([128, 24, 128], FP8, tag="xt",
                                        name="xt")
                nc.sync.dma_start(hoisted_xt0[:], xtp_d[0])
            wdma("k1")
            wdma("k2")
            wdma("q1")
            wdma("q2")
            nc.sync.dma_start(
                l1_sb[:], l1_d.rearrange("(ko p) n -> p ko n", p=128))
            nc.sync.dma_start(l2_rep[:], l2r_d[:])
            wdma("v")
            wdma("o")
            make_identity(nc, id_bf16[:])

            psB = psA if v["vride"] else psBpool

            # ---------------- phase A: projections + scores + dmats ---------
            def phase_A(mt, first=False, comb=None):
                r0 = mt * 128
                if first and hoisted_xt0 is not None:
                    xt = hoisted_xt0
                else:
                    xt = xt_p.tile([128, 24, 128], FP8, tag="xt", name="xt")
                    nc.sync.dma_start(xt[:], xtp_d[mt])
                x_nat = xnat_p.tile([128, DIM], BF16, tag="xnat", name="xnat")
                nc.sync.dma_start(x_nat[:], x_d[r0:r0 + 128, :])

                # GEMM helper: K=1024 via 4 DR chunk-pairs into [128,512]
                # psum halves. Returns the two live psum tiles.
                # i (stationary x^T chunk) outer, n (moving half) inner so
                # consecutive MM pairs share one LDWEIGHTS.
                def gemm(kbase, wname, pool, tag):
                    halves = [pool.tile([128, 512], F32, tag=tag, name=tag)
                              for _ in range(2)]
                    if v["ldw_share"]:
                        for i in range(4):
                            ksl = slice(kbase + 2 * i, kbase + 2 * i + 2)
                            wsl = slice(2 * i, 2 * i + 2)
                            for n in range(2):
                                nsl = slice(n * 512, (n + 1) * 512)
                                nc.tensor.matmul(
                                    halves[n][:], xt[:, ksl, :],
                                    w_sb[wname][:, wsl, nsl],
                                    start=(i == 0), stop=(i == 3),
                                    perf_mode=DR)
                    else:
                        for n in range(2):
                            nsl = slice(n * 512, (n + 1) * 512)
                            for i in range(4):
                                ksl = slice(kbase + 2 * i, kbase + 2 * i + 2)
                                wsl = slice(2 * i, 2 * i + 2)
                                nc.tensor.matmul(
                                    halves[n][:], xt[:, ksl, :],
                                    w_sb[wname][:, wsl, nsl],
                                    start=(i == 0), stop=(i == 3),
                                    perf_mode=DR)
                    return halves

                # r_all column layout: si*8 + q*4 + h.
                r_all = small_p.tile([128, 16], F32, tag="rall", name="rall")
                scratch = scr_p.tile([128, 512], BF16, tag="scr", name="scr")

                def scores(si, q, halves):
                    """Fused DVE stt from the Q psum halves: never evicted."""
                    kd = kdiff[f"k{si + 1}"]
                    for h in range(H):
                        ps = halves[h // 2]
                        psl = slice((h % 2) * 256, (h % 2) * 256 + 256)
                        col = si * 8 + q * 4 + h
                        nc.vector.scalar_tensor_tensor(
                            scratch[:, (h % 2) * 256:(h % 2) * 256 + 256],
                            ps[:, psl], 1.0,
                            kd[:, h * 256:(h + 1) * 256],
                            AluOp.mult, AluOp.mult,
                            accum_out=r_all[:, col:col + 1])

                vs8 = (v_p.tile([128, 2, SD], FP8, tag="vs8", name="vs8")
                       if v["c8"] else None)

                def evict_v(tok, halves):
                    if v["c8"]:
                        vt = vs8.rearrange("b kv (f c) -> b kv f c", c=512)
                        for n, ps in enumerate(halves):
                            nc.scalar.mul(vt[:, tok, n, :], ps[:],
                                          X_SCALE / QK_SCALE)
                        return vs8
                    vt = v_p.tile([128, SD], BF16, tag=f"v_{tok}",
                                  name=f"v_{tok}")
                    for n, ps in enumerate(halves):
                        nc.scalar.mul(vt[:, n * 512:(n + 1) * 512], ps[:],
                                      1.0 / QK_SCALE)
                    return vt

                kdiff = {}
                vproj = {}
                if v["fuse_proj"]:
                    # kd-pass: kd1+kd2 interleaved — each xtd chunk stationary
                    # serves 4 MMs (one LDWEIGHTS per chunk).
                    kps = {k: [psA.tile([128, 512], F32, tag="pA", name="pA")
                               for _ in range(2)] for k in ("k1", "k2")}
                    ci = 0
                    for i in range(4):
                        ksl = slice(16 + 2 * i, 16 + 2 * i + 2)
                        wsl = slice(2 * i, 2 * i + 2)
                        for kname in ("k1", "k2"):
                            for n in range(2):
                                nsl = slice(n * 512, (n + 1) * 512)
                                nc.tensor.matmul(
                                    kps[kname][n][:], xt[:, ksl, :],
                                    w_sb[kname][:, wsl, nsl],
                                    start=(i == 0), stop=(i == 3),
                                    perf_mode=DR)
                                if comb is not None:
                                    comb[ci]()
                                    ci += 1
                    for kname in ("k1", "k2"):
                        kd = kd_p.tile([128, SD], BF16, tag=f"kd_{kname}",
                                       name=f"kd_{kname}")
                        kdiff[kname] = kd
                        for n, ps in enumerate(kps[kname]):
                            nc.scalar.copy(kd[:, n * 512:(n + 1) * 512],
                                           ps[:])

                    # q-pass: per token, q1+q2(+v)+lam ride each x^T chunk
                    # stationary (5 or 7 MMs per LDWEIGHTS).
                    names = ("q1", "q2", "v") if v["vride"] else ("q1", "q2")
                    ps_lam = psL.tile([128, LH], F32, tag="pL", name="pL")
                    for tok in range(2):
                        qps = {si: [psA.tile([128, 512], F32, tag="pA",
                                             name="pA") for _ in range(2)]
                               for si in range(len(names))}
                        for i in range(4):
                            ksl = slice(tok * 8 + 2 * i, tok * 8 + 2 * i + 2)
                            wsl = slice(2 * i, 2 * i + 2)
                            for si, qname in enumerate(names):
                                for n in range(2):
                                    nsl = slice(n * 512, (n + 1) * 512)
                                    nc.tensor.matmul(
                                        qps[si][n][:], xt[:, ksl, :],
                                        w_sb[qname][:, wsl, nsl],
                                        start=(i == 0), stop=(i == 3),
                                        perf_mode=DR)
                            nc.tensor.matmul(
                                ps_lam[:], xt[:, ksl, :], l1_sb[:, ksl, :],
                                start=(tok == 0 and i == 0),
                                stop=(tok == 1 and i == 3), perf_mode=DR,
                                skip_group_check=True)
                        for si in range(2):
                            scores(si, tok, qps[si])
                        if v["vride"]:
                            vproj[tok] = evict_v(tok, qps[2])
                else:
                    # Kdiff GEMMs (stationary = host-computed xtd chunks),
                    # evicted to bf16 on ACT.
                    for kname in ("k1", "k2"):
                        halves = gemm(16, kname, psA, "pA")
                        kd = kd_p.tile([128, SD], BF16, tag=f"kd_{kname}",
                                       name=f"kd_{kname}")
                        kdiff[kname] = kd
                        for n, ps in enumerate(halves):
                            nc.scalar.copy(kd[:, n * 512:(n + 1) * 512],
                                           ps[:])
                    for si, qname in enumerate(("q1", "q2")):
                        for q in range(2):
                            halves = gemm(q * 8, qname, psA, "pA")
                            scores(si, q, halves)
                    # lam MLP hidden: standalone GEMM over all 16 chunks.
                    ps_lam = psL.tile([128, LH], F32, tag="pL", name="pL")
                    for i in range(8):
                        nc.tensor.matmul(
                            ps_lam[:], xt[:, 2 * i:2 * i + 2, :],
                            l1_sb[:, 2 * i:2 * i + 2, :],
                            start=(i == 0), stop=(i == 7), perf_mode=DR)

                # V projections (evicted on ACT; under vride they were
                # computed inside the q-pass).  c8: fp8 16*V into a shared
                # kv-interleaved tile (DoubleRow stationary for the combine).
                for tok in ([] if v["vride"] else range(2)):
                    halves = gemm(tok * 8, "v", psA, "pA")
                    vproj[tok] = evict_v(tok, halves)

                # lambda = sigmoid(relu(H') . L2/1024): relu folds into the
                # logit stt as (psum max 0) * l2, with free accumulate.
                hscr = h_p.tile([128, LH], F32, tag="hs", name="hs")
                logit = small_p.tile([128, 1], F32, tag="logit", name="logit")
                nc.vector.scalar_tensor_tensor(
                    hscr[:], ps_lam[:], 0.0, l2_rep[:], AluOp.max, AluOp.mult,
                    accum_out=logit[:])
                lam = small_p.tile([128, 1], F32, tag="lam", name="lam")
                nc.scalar.activation(lam[:], logit[:], ActFn.Sigmoid)
                eng = nc.gpsimd if v["pool_ops"] else nc.vector
                if v["nodmat"]:
                    dmats = {(q, h, kv): id_bf16 for q in range(2)
                             for h in range(H) for kv in range(2)}
                    return dict(r0=r0, x_nat=x_nat, vproj=vproj, dmats=dmats)
                u = small_p.tile([128, 1], F32, tag="u", name="u")
                eng.tensor_scalar(u[:], lam[:], -1.0, 1.0,
                                  AluOp.mult, AluOp.add)

                # a = sigmoid(SCALE/2^20 * r) for all 16 scores at once (ACT)
                a_all = small_p.tile([128, 16], F32, tag="aall", name="aall")
                nc.scalar.activation(a_all[:], r_all[:], ActFn.Sigmoid,
                                     scale=float(SCALE / QK_SCALE ** 2))

                # diff-attn weights -> diag matrices (all built on Pool)
                dmats = {}
                for q in range(2):
                    a1q = a_all[:, q * 4:(q + 1) * 4]
                    a2q = a_all[:, 8 + q * 4:8 + (q + 1) * 4]
                    t = small_p.tile([128, H], F32, tag=f"t{q}", name=f"t{q}")
                    eng.tensor_scalar_mul(t[:], a2q, lam[:])
                    w0q = small_p.tile([128, H], F32, tag=f"w0{q}",
                                       name=f"w0{q}")
                    eng.tensor_tensor(w0q[:], a1q, t[:], AluOp.subtract)
                    eng.tensor_scalar_max(w0q[:], w0q[:], 0.0)
                    w1q = small_p.tile([128, H], F32, tag=f"w1{q}",
                                       name=f"w1{q}")
                    eng.tensor_tensor(w1q[:], t[:], a1q, AluOp.subtract)
                    eng.tensor_scalar(w1q[:], w1q[:], u[:], 0.0,
                                            AluOp.add, AluOp.max)
                    for h in range(H):
                        if v["c8"]:
                            d8 = d_p.tile([128, 2, 128], FP8, tag="dmat8",
                                          name="dmat8")
                            eng.tensor_scalar_mul(
                                d8[:, 0, :], id_bf16[:], w0q[:, h:h + 1])
                            eng.tensor_scalar_mul(
                                d8[:, 1, :], id_bf16[:], w1q[:, h:h + 1])
                            dmats[(q, h)] = d8
                            continue
                        d0 = d_p.tile([128, 128], BF16, tag="dmat",
                                      name="dmat")
                        eng.tensor_scalar_mul(
                            d0[:], id_bf16[:], w0q[:, h:h + 1])
                        d1 = d_p.tile([128, 128], BF16, tag="dmat",
                                      name="dmat")
                        eng.tensor_scalar_mul(
                            d1[:], id_bf16[:], w1q[:, h:h + 1])
                        dmats[(q, h, 0)] = d0
                        dmats[(q, h, 1)] = d1

                return dict(r0=r0, x_nat=x_nat, vproj=vproj, dmats=dmats)

            # ---------------- phase B: combine + WO + output ----------------
            def make_comb(st):
                """c8 combine of tile `st` as 16 single-MM closures (at
                evicts ride the last MM of each half), for 1:1 interleave
                into the next tile's kd-pass MM stream."""
                vproj, dmats = st["vproj"], st["dmats"]
                at = {q: at_p.tile([128, 8, 128], FP8, tag=f"at{q}",
                                   name=f"at{q}") for q in range(2)}
                st["at"] = at
                btag = "pA" if v["vride"] else "pB"
                psq = {}
                vstk = vproj[0].rearrange("b kv (f c) -> b kv f c", c=128)

                def mk(half, fi, q):
                    def emit():
                        if fi == 0 and q == 0:
                            psq[half] = {
                                qq: psB.tile([128, 512], F32, tag=btag,
                                             name=btag)
                                for qq in range(2)}
                        ft = half * 4 + fi
                        nc.tensor.matmul(
                            psq[half][q][:, fi * 128:(fi + 1) * 128],
                            vstk[:, :, ft, :], dmats[(q, ft // 2)][:],
                            start=True, stop=True, perf_mode=DR,
                            skip_group_check=(fi != 0))
                        if fi == 3 and q == 1:
                            for qq in range(2):
                                nc.scalar.mul(
                                    at[qq][:, half * 4:half * 4 + 4, :],
                                    psq[half][qq][:].rearrange(
                                        "b (f c) -> b f c", c=128),
                                    float(A_SCALE / X_SCALE))
                    return emit

                return [mk(half, fi, q) for half in range(2)
                        for fi in range(4) for q in range(2)]

            def phase_B2(st):
                """WO + output for a tile whose combine already ran."""
                r0, x_nat, at = st["r0"], st["x_nat"], st["at"]
                btag = "pA" if v["vride"] else "pB"
                for q in range(2):
                    pso = [psB.tile([128, 512], F32, tag=btag, name=btag)
                           for _ in range(2)]
                    for i in range(4):
                        for n in range(2):
                            nsl = slice(n * 512, (n + 1) * 512)
                            nc.tensor.matmul(
                                pso[n][:], at[q][:, 2 * i:2 * i + 2, :],
                                w_sb["o"][:, 2 * i:2 * i + 2, nsl],
                                start=(i == 0), stop=(i == 3), perf_mode=DR)
                    for n in range(2):
                        o_sb = o_p.tile([128, 512], F32, tag="o", name="o")
                        osl = slice(q * SD + n * 512, q * SD + n * 512 + 512)
                        nc.vector.scalar_tensor_tensor(
                            o_sb[:], pso[n][:], 1.0 / O_SCALE, x_nat[:, osl],
                            AluOp.mult, AluOp.add)
                        nc.sync.dma_start(out_d[r0:r0 + 128, osl], o_sb[:])

            def phase_B(st):
                r0, x_nat = st["r0"], st["x_nat"]
                vproj, dmats = st["vproj"], st["dmats"]
                # A_q^T via diag matmuls, 4 ftiles packed per psum bank.
                # (ft, kv) stationary V chunk serves both q's MMs back to
                # back (different dmats/psums) so LDWEIGHTS count halves.
                at = {q: at_p.tile([128, 8, 128], FP8, tag=f"at{q}",
                                   name=f"at{q}") for q in range(2)}
                btag = "pA" if v["vride"] else "pB"
                for half in range(2):
                    psq = {q: psB.tile([128, 512], F32, tag=btag, name=btag)
                           for q in range(2)}
                    for fi in range(4):
                        ft = half * 4 + fi
                        h = ft // 2
                        fsl = slice(ft * 128, (ft + 1) * 128)
                        if v["c8"]:
                            vstk = vproj[0].rearrange(
                                "b kv (f c) -> b kv f c", c=128)
                            for q in range(2):
                                nc.tensor.matmul(
                                    psq[q][:, fi * 128:(fi + 1) * 128],
                                    vstk[:, :, ft, :], dmats[(q, h)][:],
                                    start=True, stop=True, perf_mode=DR,
                                    skip_group_check=(fi != 0))
                            continue
                        for kv in range(2):
                            for q in range(2):
                                nc.tensor.matmul(
                                    psq[q][:, fi * 128:(fi + 1) * 128],
                                    vproj[kv][:, fsl], dmats[(q, h, kv)][:],
                                    start=(kv == 0), stop=(kv == 1),
                                    skip_group_check=(fi != 0))
                    at_scale = A_SCALE / X_SCALE if v["c8"] else A_SCALE
                    for q in range(2):
                        nc.scalar.mul(
                            at[q][:, half * 4:half * 4 + 4, :],
                            psq[q][:].rearrange("b (f c) -> b f c", c=128),
                            float(at_scale))

                # out_q = (A'_q @ WO')/512 + tokens_q   (fp8 DoubleRow);
                # i (stationary A^T chunk) outer, n inner: shared LDWEIGHTS.
                for q in range(2):
                    pso = [psB.tile([128, 512], F32, tag=btag, name=btag)
                           for _ in range(2)]
                    for i in range(4):
                        for n in range(2):
                            nsl = slice(n * 512, (n + 1) * 512)
                            nc.tensor.matmul(
                                pso[n][:], at[q][:, 2 * i:2 * i + 2, :],
                                w_sb["o"][:, 2 * i:2 * i + 2, nsl],
                                start=(i == 0), stop=(i == 3), perf_mode=DR)
                    for n in range(2):
                        o_sb = o_p.tile([128, 512], F32, tag="o", name="o")
                        osl = slice(q * SD + n * 512, q * SD + n * 512 + 512)
                        nc.vector.scalar_tensor_tensor(
                            o_sb[:], pso[n][:], 1.0 / O_SCALE, x_nat[:, osl],
                            AluOp.mult, AluOp.add)
                        nc.sync.dma_start(out_d[r0:r0 + 128, osl], o_sb[:])

            # ---------------- main loop (A/B software pipeline) -------------
            rep_cm = (tc.For_i(0, hw_repeats, 1) if hw_repeats > 1
                      else contextlib.nullcontext())
            with rep_cm:
                use_cmix = (v["cmix"] and v["c8"] and v["fuse_proj"]
                            and v["pipeline"] and not v["nob"])
                pending = None
                for mt_rep in range(n_mtiles * repeats):
                    mt = mt_rep % n_mtiles
                    comb = (make_comb(pending)
                            if use_cmix and pending is not None else None)
                    st = phase_A(mt, first=(mt_rep == 0), comb=comb)
                    if v["nob"]:
                        continue
                    if not v["pipeline"]:
                        phase_B(st)
                        continue
                    if pending is not None:
                        if comb is not None:
                            phase_B2(pending)
                        else:
                            phase_B(pending)
                    pending = st
                if pending is not None:
                    phase_B(pending)

    split_excess_waits(nc)
    return nc


_NC_CACHE = {}


def _get_nc(n_samples):
    if n_samples not in _NC_CACHE:
        _NC_CACHE[n_samples] = build_kernel(n_samples)
    return _NC_CACHE[n_samples]


def host_prep(inputs, n_samples=B_CORE):
    """Host-side shard + dtype/layout prep. Returns in_maps for 8 cores."""
    x = np.ascontiguousarray(np.asarray(inputs["x"], dtype=np.float32))
    assert x.shape[0] == N_CORES * n_samples and x.shape[1] == DIM
    f8 = ml_dtypes.float8_e4m3fn
    bf16 = ml_dtypes.bfloat16
    ws = {}
    for k in ("WQ1_w", "WK1_w", "WQ2_w", "WK2_w", "WV_w", "WO_w", "L1_w"):
        ws[k] = np.ascontiguousarray(
            (np.asarray(inputs[k], dtype=np.float32) * W_SCALE).astype(f8))
    l2rep = np.ascontiguousarray(
        np.broadcast_to(np.asarray(inputs["L2_w"], dtype=np.float32)
                        .reshape(1, LH) / QK_SCALE, (128, LH)))
    n_mtiles = n_samples // 128
    # scale+cast once (contiguous, fast), then one big strided transpose in
    # fp8: [core, mt, b, ft, p] -> [core, mt, p, ft, b]; then append the
    # host-computed token difference (T0-T1)^T as ftiles 16..23.
    xb = (x * X_SCALE).astype(f8).reshape(N_CORES, n_mtiles, 128, 16, 128)
    xt = xb.transpose(0, 1, 4, 3, 2)  # [core, mt, p, ft, b]
    xtd = (xt[:, :, :, 0:8, :].astype(np.float32)
           - xt[:, :, :, 8:16, :].astype(np.float32)).astype(f8)
    xtp_all = np.ascontiguousarray(
        np.concatenate([xt, xtd], axis=3))  # [core, mt, p, 24, b]
    xb16 = x.astype(bf16).reshape(N_CORES, n_samples, DIM)
    in_maps = []
    for c in range(N_CORES):
        m = {"xb": xb16[c], "xtp": xtp_all[c], "L2r": l2rep}
        m.update(ws)
        in_maps.append(m)
    return in_maps


def kernel(**inputs):
    from concourse.bass_utils import run_bass_kernel_spmd

    nc = _get_nc(B_CORE)
    in_maps = host_prep(inputs, B_CORE)
    res = run_bass_kernel_spmd(nc, in_maps, core_ids=list(range(N_CORES)))
    return np.concatenate([res.results[c]["out"] for c in range(N_CORES)],
                          axis=0)
